# revision 26
# baseline (speedup 1.0000x reference)
"""Trainium2 Bass kernel for ConvGPTAttention (dense transformer attention block).

Sharding: tensor-parallel by head groups across 8 NeuronCores.
Core j owns q heads {2j, 2j+1} and kv head j (GQA maps q head h -> kv head h//2,
so each core's attention is fully local). Wqkv is column-sharded, Wo is
row-sharded; the 8 partial o_proj outputs are summed on the host (the
"all-reduce" of RowParallelLinear, done at unshard time).

Numerics/dtype plan (validated against the reference in fp64 emulation):
  - QKV GEMM: fp8e4m3 DoubleRow matmuls with full error compensation:
    qkv = (X_hi + X_lo) @ W_hi + X_hi @ W_lo  (X_lo/W_lo are fp8 quantization
    residuals). W pre-scaled by 64 on the host; RMSNorm is scale-invariant.
  - q/k post-norm+rope stored fp16; attention scores in fp16 matmuls.
  - Causal masking is done on the PE: an extra matmul accumulates a
    [128,128] NEG upper-triangle block (maskT^T @ I) into the diagonal
    score blocks, so no DVE/Pool masking pass is needed.
  - softmax: exp on ScalarE with scale=1/sqrt(128), bias=-2ln2 (keeps fp8
    range safe); es stored fp8; denominator via a [128,2,1]=32.0 DoubleRow
    ones-matmul sharing the es stream; AV via fp8 DoubleRow over s-block
    pairs (v stored fp8 at 32x scale).
  - o_proj in fp16; partial outputs written fp16, summed on host in fp32.
"""

import numpy as np
import ml_dtypes
from contextlib import ExitStack

import concourse.bacc as bacc
import concourse.mybir as mybir
import concourse.tile as tile
from concourse.bass_utils import run_bass_kernel_spmd

P = 128
T = 2048
H = 2048
N_HEADS = 16
N_KV = 8
HD = 128
EPS = 1e-6
THETA = 10000.0
SCALE = HD ** -0.5
NEG = -60000.0
EXP_BIAS = -2.0 * float(np.log(2.0))

F32 = mybir.dt.float32
F16 = mybir.dt.float16
F8 = mybir.dt.float8e4
AF = mybir.ActivationFunctionType
ALU = mybir.AluOpType
DR = mybir.MatmulPerfMode.DoubleRow

NP_F8 = ml_dtypes.float8_e4m3

N_CORES = 8
N_TT = 16        # t-tiles of 128 tokens
N_TB = 4         # t-blocks of 512 tokens


def _build_nc():
    nc = bacc.Bacc("TRN2", target_bir_lowering=False, debug=False)

    xhl = nc.dram_tensor("xhl", [P, 4, 16, 2, 512], F8, kind="ExternalInput")
    whi = nc.dram_tensor("whi", [P, 16, 512], F8, kind="ExternalInput")
    wlo = nc.dram_tensor("wlo", [P, 16, 512], F8, kind="ExternalInput")
    wo = nc.dram_tensor("wo", [256, H], F16, kind="ExternalInput")
    ab = nc.dram_tensor("ab", [T, 4, 3, 64], F16, kind="ExternalInput")
    maskt = nc.dram_tensor("maskt", [P, P], F16, kind="ExternalInput")
    ident = nc.dram_tensor("ident", [P, P], F16, kind="ExternalInput")
    ones = nc.dram_tensor("ones", [P, 2, 16], F8, kind="ExternalInput")
    out = nc.dram_tensor("out", [T, H], F16, kind="ExternalOutput")

    with ExitStack() as top:
        tc = top.enter_context(tile.TileContext(nc))
        pers = top.enter_context(tc.tile_pool(name="pers", bufs=1))

        ident_sb = pers.tile([P, P], F16, tag="ident")
        maskt_sb = pers.tile([P, P], F16, tag="maskt")
        ones_sb = pers.tile([P, 2, 16], F8, tag="ones")
        wo_sb = pers.tile([P, 2, H], F16, tag="wo")
        eps_sb = pers.tile([P, 1], F32, tag="eps")
        nc.vector.memset(eps_sb[:], 256.0 * EPS)
        ebias_sb = pers.tile([P, 1], F32, tag="ebias")
        nc.vector.memset(ebias_sb[:], EXP_BIAS)

        # persistent activations
        qkT = pers.tile([P, 3, T], F16, tag="qkT")      # [d, (q0,q1,k), t]
        v8 = pers.tile([P, N_TT, P], F8, tag="v8")      # [s_in, tt, d] (32x)
        v16 = pers.tile([P, 4, P], F16, tag="v16")      # tiles 0-3, f16 (32x)
        ones16 = pers.tile([P, 1], F16, tag="ones16")
        nc.vector.memset(ones16[:], 32.0)
        attn16 = pers.tile([P, 2, T], F16, tag="attn")  # [d, qh, t] normalized
        expp = top.enter_context(tc.tile_pool(name="expp", bufs=5))
        expp16 = top.enter_context(tc.tile_pool(name="expp16", bufs=4))
        woven_es = {}

        def emit_pair_scores(tb, qh, jp, pss_pool):
            """scores + causal mask + exp for s-block pair jp of (tb, qh).
            Returns (es_tile, o_region)."""
            q0 = tb * 512
            off = jp - 2 * tb       # 0,1 for diagonal pairs
            if tb == 0:
                # tb0 rows have tiny softmax support: keep es in fp16
                es = expp16.tile([P, 2, 512], F16, tag="es16",
                                 name=f"es_{tb}_{qh}_{jp}")
            else:
                es = expp.tile([P, 2, 512], F8, tag="es",
                               name=f"es_{tb}_{qh}_{jp}")
            ps_p = pss_pool.tile([P, 2, 512], F32, tag="sp",
                                 name=f"sp_{tb}_{qh}_{jp}")
            if 0 <= off < 2:
                o = 256 * off
                # zero the never-written leading es regions
                if off == 0:
                    nc.gpsimd.memset(es[:, 1, 0:128], 0.0)
                else:
                    nc.gpsimd.memset(es[:, 0, 0:256], 0.0)
                    nc.gpsimd.memset(es[:, 1, 0:384], 0.0)
                for i in range(2):
                    oi = o + 128 * i
                    sb = 4 * tb + 2 * off + i
                    nc.tensor.matmul(
                        ps_p[:, i, oi:512],
                        qkT[:, 2, sb * P:(sb + 1) * P],
                        qkT[:, qh, q0 + oi:q0 + 512],
                        start=True,
                        stop=False,
                    )
                    # causal triangle via PE: += maskT^T @ I
                    nc.tensor.matmul(
                        ps_p[:, i, oi:oi + 128],
                        maskt_sb[:],
                        ident_sb[:],
                        start=False,
                        stop=True,
                    )
                    nc.scalar.activation(
                        es[:, i, oi:512], ps_p[:, i, oi:512],
                        AF.Exp, scale=SCALE, bias=ebias_sb[:],
                    )
                if tb == 0 and off == 0:
                    # t=0 row: single-entry softmax, make exact
                    nc.gpsimd.memset(es[0:1, 0, 0:1], 1.0)
                return es, o
            for i in range(2):
                sb = 2 * jp + i
                nc.tensor.matmul(
                    ps_p[:, i, :],
                    qkT[:, 2, sb * P:(sb + 1) * P],
                    qkT[:, qh, q0:q0 + 512],
                    start=True,
                    stop=True,
                )
            nc.scalar.activation(
                es[:], ps_p[:], AF.Exp, scale=SCALE, bias=ebias_sb[:],
            )
            return es, 0

        # ---------------- Phase A: QKV + norm + rope + transpose ------------
        with ExitStack() as pa_ctx:
            wp = pa_ctx.enter_context(tc.tile_pool(name="wp", bufs=1))
            xp = pa_ctx.enter_context(tc.tile_pool(name="xp", bufs=2))
            pa = pa_ctx.enter_context(tc.tile_pool(name="pa", bufs=2))
            psa = pa_ctx.enter_context(tc.tile_pool(name="psa", bufs=3, space="PSUM"))
            pst = pa_ctx.enter_context(tc.tile_pool(name="pst", bufs=2, space="PSUM"))
            pssw = pa_ctx.enter_context(tc.tile_pool(name="pssw", bufs=1, space="PSUM"))

            whi_sb = wp.tile([P, 16, 512], F8, tag="whi")
            wlo_sb = wp.tile([P, 16, 512], F8, tag="wlo")
            x_first = xp.tile([P, 16, 2, 512], F8, tag="x", name="x_ts0")
            # DMA order matches the chunk-major warmup consumption order
            for cg in range(4):
                nc.sync.dma_start(x_first[:, 4 * cg:4 * cg + 4],
                                  xhl[:, 0, 4 * cg:4 * cg + 4])
                nc.sync.dma_start(whi_sb[:, 4 * cg:4 * cg + 4],
                                  whi[:, 4 * cg:4 * cg + 4])
                nc.sync.dma_start(wlo_sb[:, 4 * cg:4 * cg + 4],
                                  wlo[:, 4 * cg:4 * cg + 4])
            nc.sync.dma_start(ident_sb[:], ident[:])
            nc.sync.dma_start(maskt_sb[:], maskt[:])
            nc.sync.dma_start(ones_sb[:], ones[:])

            qkn_done = {}

            def emit_transposes(go):
                qkn_o = qkn_done.pop(go)
                for ttl in range(2):
                    tt = 2 * go + ttl
                    ps_t = pst.tile([P, 3, P], F16, tag="pst",
                                    name=f"pst_{tt}")
                    for h in range(3):
                        nc.tensor.transpose(
                            ps_t[:, h, :], qkn_o[:, ttl, h, :], ident_sb[:]
                        )
                    nc.vector.tensor_copy(
                        qkT[:, :, tt * P:(tt + 1) * P], ps_t[:]
                    )

            for ts in range(4):          # 512-token superblocks
                if ts == 0:
                    x_sb = x_first
                else:
                    x_sb = xp.tile([P, 16, 2, 512], F8, tag="x",
                                   name=f"x_ts{ts}")
                    nc.sync.dma_start(x_sb[:, 0:4], xhl[:, ts, 0:4])
                ab_sb = xp.tile([P, 4, 4, 3, 64], F16, tag="ab")
                if ts > 0:
                    nc.sync.dma_start(x_sb[:, 4:10], xhl[:, ts, 4:10])
                nc.sync.dma_start(ab_sb[:], ab[ts * 512:(ts + 1) * 512]
                                  .rearrange("(a p) c h d -> p a c h d", p=P))
                if ts > 0:
                    nc.sync.dma_start(x_sb[:, 10:16], xhl[:, ts, 10:16])

                def emit_whi_mms(ps_a, lsl, c_lo, c_hi):
                    for c in range(c_lo, c_hi):
                        nc.tensor.matmul(
                            ps_a[:],
                            x_sb[:, c, :, lsl],
                            whi_sb[:, c, None, :].to_broadcast((P, 2, 512)),
                            start=(c == 0),
                            stop=False,
                            perf_mode=DR,
                        )

                def emit_wlo_mms(ps_a, lsl, cp_lo, cp_hi):
                    for cp in range(cp_lo, cp_hi):
                        nc.tensor.matmul(
                            ps_a[:],
                            x_sb[:, 2 * cp:2 * cp + 2, 0, lsl],
                            wlo_sb[:, 2 * cp:2 * cp + 2, :],
                            start=False,
                            stop=(cp == 7),
                            perf_mode=DR,
                        )

                pre_psa = {}
                if ts == 0:
                    # warmup: chunk-major over the first 3 tiles so each
                    # arriving X/W chunk slab feeds matmuls immediately
                    for ti in range(3):
                        pre_psa[ti] = psa.tile([P, 512], F32, tag="psa",
                                               name=f"psa_w{ti}")
                    for cg in range(4):
                        for ti in range(3):
                            emit_whi_mms(pre_psa[ti], slice(ti * P, (ti + 1) * P),
                                         4 * cg, 4 * cg + 4)
                        for ti in range(3):
                            emit_wlo_mms(pre_psa[ti], slice(ti * P, (ti + 1) * P),
                                         2 * cg, 2 * cg + 2)

                for tg in range(2):      # groups of 2 t-tiles (256 tokens)
                    g = ts * 2 + tg
                    qk16 = pa.tile([P, 2, 3, P], F16, tag="qk16")
                    sq = pa.tile([P, 2, 3, P], F16, tag="sq")
                    qkn = pa.tile([P, 2, 3, P], F16, tag="qkn",
                                  name=f"qkn_{g}")
                    m1 = pa.tile([P, 2, 3, 64], F16, tag="m1")
                    m2 = pa.tile([P, 2, 3, 64], F16, tag="m2")
                    ss = pa.tile([P, 2, 3], F32, tag="ss")
                    sr = pa.tile([P, 2, 3], F32, tag="sr")
                    si = pa.tile([P, 2, 3], F32, tag="si")

                    for ttl in range(2):
                        tt = ts * 4 + tg * 2 + ttl
                        tl_local = tg * 2 + ttl
                        tsl = slice(tl_local * P, (tl_local + 1) * P)
                        if ts == 0 and tl_local < 3:
                            ps_a = pre_psa[tl_local]
                        else:
                            ps_a = psa.tile([P, 512], F32, tag="psa")
                            emit_whi_mms(ps_a, tsl, 0, 16)
                            emit_wlo_mms(ps_a, tsl, 0, 8)
                        # v: psum(64x) -> fp8 at 32x
                        nc.scalar.activation(
                            v8[:, tt, :], ps_a[:, 384:512], AF.Copy, scale=0.5
                        )
                        if tt < 4:
                            # f16 copy for the high-precision tb0 path
                            nc.scalar.activation(
                                v16[:, tt, :], ps_a[:, 384:512], AF.Copy,
                                scale=0.5
                            )
                        # q/k copy to fp16 at 16x scale (f16^2-safe)
                        nc.scalar.activation(
                            qk16[:, ttl], ps_a[:, 0:384], AF.Copy, scale=0.25
                        )
                        # sq = (0.25*x)^2 = 256*q_u^2
                        nc.scalar.activation(
                            sq[:, ttl], ps_a[:, 0:384], AF.Square, scale=0.25
                        )
                        nc.vector.tensor_reduce(
                            ss[:, ttl], sq[:, ttl], axis=mybir.AxisListType.X,
                            op=ALU.add,
                        )

                    # rms(16x) = sqrt(ss/128 + 256*eps)
                    nc.scalar.activation(
                        sr[:], ss[:], AF.Sqrt, scale=1.0 / 128.0, bias=eps_sb[:]
                    )
                    nc.vector.reciprocal(si[:], sr[:])

                    # neox rope in fp16 (DVE 2x mode): tables fold norm weight
                    abg = ab_sb[:, 2 * tg:2 * tg + 2]    # [P, 2, 4, 3, 64]
                    x1 = qk16[:, :, :, 0:64]
                    x2 = qk16[:, :, :, 64:128]
                    nc.vector.tensor_mul(m1[:], x1, abg[:, :, 0])
                    nc.vector.tensor_mul(m2[:], x2, abg[:, :, 1])
                    nc.vector.tensor_sub(qkn[:, :, :, 0:64], m1[:], m2[:])
                    nc.vector.tensor_mul(m1[:], x2, abg[:, :, 2])
                    nc.vector.tensor_mul(m2[:], x1, abg[:, :, 3])
                    nc.vector.tensor_add(qkn[:, :, :, 64:128], m1[:], m2[:])
                    # apply 1/rms (broadcast over d)
                    nc.vector.tensor_mul(
                        qkn[:], qkn[:],
                        si[:, :, :, None].to_broadcast((P, 2, 3, P)),
                    )
                    qkn_done[g] = qkn

                    # transposes lag one group so the PE never waits on the
                    # rope chain of the current group
                    if g > 0:
                        emit_transposes(g - 1)
            # weave tb0 + tb1-qh0 scores+exp into the last rope chain's shadow
            for qh_ in range(2):
                for jp_ in (0, 1):
                    woven_es[(0, qh_, jp_)] = emit_pair_scores(0, qh_, jp_, pssw)
            emit_transposes(7)

        # ---------------- Phase B: attention + o_proj -----------------------
        nc.sync.dma_start(wo_sb[:], wo.rearrange("(q p) h -> p q h", p=P))
        with ExitStack() as pb_ctx:
            nrm = pb_ctx.enter_context(tc.tile_pool(name="nrm", bufs=2))
            outp = pb_ctx.enter_context(tc.tile_pool(name="outp", bufs=4))
            pss = pb_ctx.enter_context(tc.tile_pool(name="pss", bufs=2, space="PSUM"))
            psat = pb_ctx.enter_context(tc.tile_pool(name="psat", bufs=1, space="PSUM"))
            psse = pb_ctx.enter_context(tc.tile_pool(name="psse", bufs=1, space="PSUM"))
            pso = pb_ctx.enter_context(tc.tile_pool(name="pso", bufs=2, space="PSUM"))

            o_stage = {}
            o_count = 0

            def emit_oproj_unit(tt, hb, split=False):
                nonlocal o_count
                if hb == 0:
                    o_stage[tt] = outp.tile(
                        [P, 4, 512], F16, tag="ostg", name=f"ostg_{tt}"
                    )
                ps_o = pso.tile([P, 512], F32, tag="o")
                for hh in range(2):
                    nc.tensor.matmul(
                        ps_o[:],
                        attn16[:, hh, tt * P:(tt + 1) * P],
                        wo_sb[:, hh, hb * 512:(hb + 1) * 512],
                        start=(hh == 0),
                        stop=(hh == 1),
                    )
                dst = o_stage[tt][:, hb, :]
                if split:
                    # tail flush: spread one copy over both psum-capable engines
                    nc.scalar.copy(dst[:, 0:256], ps_o[:, 0:256])
                    nc.vector.tensor_copy(dst[:, 256:512], ps_o[:, 256:512])
                else:
                    # early units land where DVE is congested (A->B boundary),
                    # late units where ScalarE (exp) is busiest
                    r = o_count
                    o_count += 1
                    if r < 24:
                        eng = (nc.scalar.copy, nc.vector.tensor_copy)[r % 2]
                    else:
                        eng = (nc.scalar.copy, nc.vector.tensor_copy,
                               nc.vector.tensor_copy)[r % 3]
                    eng(dst, ps_o[:])
                if split:
                    if hb in (1, 3):
                        nc.sync.dma_start(
                            out[tt * P:(tt + 1) * P, (hb - 1) * 512:(hb + 1) * 512],
                            o_stage[tt][:, hb - 1:hb + 1].rearrange(
                                "p a c -> p (a c)"),
                        )
                elif hb == 3:
                    nc.sync.dma_start(
                        out[tt * P:(tt + 1) * P, :],
                        o_stage[tt][:].rearrange("p a c -> p (a c)"),
                    )

            oproj_queue = []

            def queue_oproj(tbo):
                for ttl in range(4):
                    for hb in range(4):
                        oproj_queue.append((4 * tbo + ttl, hb))

            for tb in range(N_TB):
                q0 = tb * 512
                npair = 2 * (tb + 1)
                for qh in range(2):
                    ps_at = psat.tile([P, 512], F32, tag="at")
                    ps_se = psse.tile([1, 512], F32, tag="se")

                    # pair order: diagonal pairs first, then off-diagonal
                    pair_order = [2 * tb, 2 * tb + 1] + list(range(2 * tb))

                    def emit_avse(jp, pi, o):
                        es = es_tiles[jp]
                        if tb == 0:
                            # f16 path: per-block matmuls with f16 es/v
                            for i in range(2):
                                oi = o + 128 * i if jp < 2 else o
                                nc.tensor.matmul(
                                    ps_at[:, oi:512],
                                    v16[:, 2 * jp + i, :],
                                    es[:, i, oi:512],
                                    start=(pi == 0 and i == 0),
                                    stop=(pi == npair - 1 and i == 1),
                                    skip_group_check=True,
                                )
                                nc.tensor.matmul(
                                    ps_se[:, oi:512],
                                    ones16[:],
                                    es[:, i, oi:512],
                                    start=(pi == 0 and i == 0),
                                    stop=(pi == npair - 1 and i == 1),
                                    skip_group_check=True,
                                )
                            return
                        nc.tensor.matmul(
                            ps_at[:, o:512],
                            v8[:, 2 * jp:2 * jp + 2, :],
                            es[:, :, o:512],
                            start=(pi == 0),
                            stop=(pi == npair - 1),
                            perf_mode=DR,
                            skip_group_check=True,
                        )
                        nc.tensor.matmul(
                            ps_se[:, o:512],
                            ones_sb[:, :, 0:1],
                            es[:, :, o:512],
                            start=(pi == 0),
                            stop=(pi == npair - 1),
                            perf_mode=DR,
                            skip_group_check=True,
                        )

                    es_tiles = {}
                    pending = []
                    for pi, jp in enumerate(pair_order):
                        if (tb, qh, jp) in woven_es:
                            es, o_region = woven_es.pop((tb, qh, jp))
                        else:
                            es, o_region = emit_pair_scores(tb, qh, jp, pss)
                        es_tiles[jp] = es
                        pending.append((jp, pi, o_region))
                        if len(pending) > 2:
                            emit_avse(*pending.pop(0))
                        for _ in range(2):
                            if oproj_queue:
                                emit_oproj_unit(*oproj_queue.pop(0))
                    for item in pending:
                        emit_avse(*item)

                    # softmax denominator -> broadcast -> normalize (fp16 out)
                    inv_sb = nrm.tile([1, 512], F32, tag="inv")
                    nc.vector.reciprocal(inv_sb[:], ps_se[:])
                    bc = nrm.tile([P, 512], F32, tag="bc")
                    nc.gpsimd.partition_broadcast(bc[:], inv_sb[0:1, :])
                    nc.vector.tensor_mul(
                        attn16[:, qh, q0:q0 + 512], ps_at[:], bc[:]
                    )
                while oproj_queue:
                    emit_oproj_unit(*oproj_queue.pop(0))
                queue_oproj(tb)
            while oproj_queue:
                emit_oproj_unit(*oproj_queue.pop(0), split=True)

    nc.compile()
    return nc


_NC_CACHE = {}


def _get_nc():
    if "nc" not in _NC_CACHE:
        _NC_CACHE["nc"] = _build_nc()
    return _NC_CACHE["nc"]


def kernel(positions, hidden_states, Wqkv, Wo, q_norm_w, k_norm_w):
    positions = np.asarray(positions)
    out_dtype = np.asarray(hidden_states).dtype
    hs = np.asarray(hidden_states, dtype=np.float32)
    Wqkv = np.asarray(Wqkv, dtype=np.float32)
    Wo = np.asarray(Wo, dtype=np.float32)
    qw = np.asarray(q_norm_w, dtype=np.float32)
    kw = np.asarray(k_norm_w, dtype=np.float32)

    # ----- host-side input prep -----
    xt = np.ascontiguousarray(hs.T)                       # [H, T]
    xhi = xt.astype(NP_F8)
    xlo = (xt - xhi.astype(np.float32)).astype(NP_F8)
    # [p, ts, c, l, t']
    xhl = np.empty((P, 4, 16, 2, 512), dtype=NP_F8)
    for l, arr in enumerate((xhi, xlo)):
        r = arr.reshape(16, P, 4, 512)                    # [c, p, ts, t']
        xhl[:, :, :, l, :] = r.transpose(1, 2, 0, 3)

    inv_freq = (1.0 / (THETA ** (np.arange(0, HD, 2, dtype=np.float32) / HD)))
    freqs = positions.astype(np.float32)[:, None] * inv_freq[None, :]
    cos = np.cos(freqs).astype(np.float32)
    sin = np.sin(freqs).astype(np.float32)

    def ab_tables(wvec):
        a1 = cos * wvec[None, :64]
        b1 = sin * wvec[None, 64:]
        a2 = cos * wvec[None, 64:]
        b2 = sin * wvec[None, :64]
        return np.stack([a1, b1, a2, b2], axis=1)         # [T, 4, 64]

    abq = ab_tables(qw)
    abk = ab_tables(kw)
    ab = np.ascontiguousarray(
        np.stack([abq, abq, abk], axis=2), dtype=np.float16
    )                                                     # [T, 4, 3, 64]

    # causal triangle block: maskT[c, p] = NEG if c < p else 0
    c_i = np.arange(P)
    maskt = np.where(c_i[:, None] < c_i[None, :], NEG, 0.0).astype(np.float16)
    ident = np.eye(P, dtype=np.float16)
    ones = np.full((P, 2, 16), 32.0, dtype=NP_F8)

    q_size = N_HEADS * HD
    kv_size = N_KV * HD
    in_maps = []
    for j in range(N_CORES):
        qs = slice(2 * j * HD, (2 * j + 2) * HD)
        ks = slice(q_size + j * HD, q_size + (j + 1) * HD)
        vs = slice(q_size + kv_size + j * HD, q_size + kv_size + (j + 1) * HD)
        wj = np.concatenate(
            [Wqkv[:, qs], Wqkv[:, ks], Wqkv[:, vs]], axis=1
        ) * 64.0                                          # [H, 512]
        whi_f = wj.astype(NP_F8)
        wlo_f = (wj - whi_f.astype(np.float32)).astype(NP_F8)
        whi = np.ascontiguousarray(
            whi_f.reshape(16, P, 512).transpose(1, 0, 2))
        wlo = np.ascontiguousarray(
            wlo_f.reshape(16, P, 512).transpose(1, 0, 2))
        woj = np.ascontiguousarray(Wo[qs, :], dtype=np.float16)
        in_maps.append(
            {
                "xhl": xhl,
                "whi": whi,
                "wlo": wlo,
                "wo": woj,
                "ab": ab,
                "maskt": maskt,
                "ident": ident,
                "ones": ones,
            }
        )

    nc = _get_nc()
    res = run_bass_kernel_spmd(nc, in_maps, core_ids=list(range(N_CORES)))

    acc = res.results[0]["out"].astype(np.float32)
    for j in range(1, N_CORES):
        acc += res.results[j]["out"].astype(np.float32)
    return acc.astype(out_dtype, copy=False)
